# revision 5
# baseline (speedup 1.0000x reference)
"""AWQ W4-packed linear layer (y = (x * x_inv_s) @ dequant(w).T + bias)
on 8 Trainium2 NeuronCores, tensor-parallel over out_features (no
collectives).

Per core (1376 of the 11008 out_features):
  * The kernel is tensor-engine bound: 64 token tiles x 32 groups x 1376
    outs of bf16 matmul = 2.82M PE cycles (~1.18 ms at 2.4 GHz). All
    other work is scheduled to hide under it; the previous version lost
    ~0.5 ms to a serial weight-dequant prologue.
  * Weight prep (pipelined, ~0.1 ms): per 128-row tile, one 8 KB/p int32
    DMA, two wide DVE shift ops sign-extend nibbles to int8, 32 fused
    scalar_tensor_tensor ops (split DVE/GpSimd) apply (w*s)*x_inv -> bf16,
    PE transposes 8 groups into one PSUM bank, one wide ACT copy
    evacuates to the SBUF-resident W^T [128 in, 32 g, out].
  * Activations: per 512-token chunk, a SWDGE cast-DMA converts x
    fp32->bf16 into DRAM staging; 32 xbar DMA-transposes (single HWDGE
    ring) produce X^T. Chunks 0-1 are issued at t=0 so the x pipe warms
    up concurrently with weight prep.
  * Matmul: per 128-token tile, three PSUM banks (N=512/512/352)
    accumulate 32 bf16 matmuls each; bias is added on PSUM evacuation
    (DVE, bf16 out) and a cast-DMA store writes the fp32 output shard.
"""
import sys

import numpy as np
import ml_dtypes

try:
    import concourse.bass as bass
except ImportError:  # fallback if PYTHONPATH lacks the repo
    for p in ("/opt/trn_rl_repo", "/root/.axon_site/_ro/trn_rl_repo"):
        if p not in sys.path:
            sys.path.append(p)
    import concourse.bass as bass

import concourse.bacc as bacc
import concourse.tile as tile
import concourse.mybir as mybir
from concourse.bass_utils import run_bass_kernel_spmd

F32 = mybir.dt.float32
BF16 = mybir.dt.bfloat16
I32 = mybir.dt.int32
I8 = mybir.dt.int8

OUT_F, IN_F, GROUP = 11008, 4096, 128
NG = IN_F // GROUP            # 32 quantization groups
NT = 8192                     # B*S tokens
N_CORES = 8
OS = OUT_F // N_CORES         # 1376 out features per core
TCH = 512                     # tokens per chunk
N_CHUNK = NT // TCH           # 16
O_TILES = (OS + 127) // 128   # 11 (last has 96 rows)
OC_SLICES = [(0, 512), (512, 1024), (1024, 1376)]

_NC_CACHE = {}


def build_nc():
    if "nc" in _NC_CACHE:
        return _NC_CACHE["nc"]
    nc = bacc.Bacc("TRN2", target_bir_lowering=False, debug=False,
                   num_devices=N_CORES)

    x = nc.dram_tensor("x", [NT, IN_F], F32, kind="ExternalInput").ap()
    wq = nc.dram_tensor("wq", [OS, NG * 64], I32, kind="ExternalInput").ap()
    ws = nc.dram_tensor("ws", [OS, NG], F32, kind="ExternalInput").ap()
    xinvb = nc.dram_tensor("xinvb", [128, IN_F], BF16,
                           kind="ExternalInput").ap()
    biasb = nc.dram_tensor("biasb", [128, OS], F32, kind="ExternalInput").ap()
    ident = nc.dram_tensor("ident", [128, 128], BF16,
                           kind="ExternalInput").ap()
    y = nc.dram_tensor("y", [NT, OS], F32, kind="ExternalOutput").ap()

    with tile.TileContext(nc) as tc:
        with (
            tc.tile_pool(name="cpool", bufs=1) as cpool,
            tc.tile_pool(name="wpool", bufs=2) as wpool,
            tc.tile_pool(name="xpool", bufs=2) as xpool,
            tc.tile_pool(name="dpool", bufs=2,
                         space=bass.MemorySpace.DRAM) as dpool,
            tc.tile_pool(name="ypool", bufs=2) as ypool,
            tc.tile_pool(name="wps", bufs=2,
                         space=bass.MemorySpace.PSUM) as wps,
            tc.tile_pool(name="aps", bufs=6,
                         space=bass.MemorySpace.PSUM) as aps,
        ):
            # ---------------- persistent tiles ----------------
            WTs = [cpool.tile([128, NG, o1 - o0], BF16, name=f"WT{i}")
                   for i, (o0, o1) in enumerate(OC_SLICES)]   # [i, g, o]
            id_sb = cpool.tile([128, 128], BF16)
            nc.scalar.dma_start(id_sb[:], ident[:])
            xinv_sb = cpool.tile([128, IN_F], BF16)
            nc.scalar.dma_start(xinv_sb[:], xinvb[:])
            bias_sb = cpool.tile([128, OS], F32)
            nc.scalar.dma_start(bias_sb[:], biasb[:])

            # ---------------- x pipeline helpers ----------------------
            XTs = {}
            xbss = {}

            def emit_cast(ch):
                t0 = ch * TCH
                xbs = dpool.tile([TCH, IN_F], BF16, tag="xbs",
                                 name=f"xbs{ch}")
                xbss[ch] = xbs
                nc.gpsimd.dma_start(xbs[:], x[t0:t0 + TCH, :])  # f32->bf16

            def emit_transposes(ch):
                xbs = xbss.pop(ch)
                XT = xpool.tile([128, NG, TCH], BF16, tag="xt",
                                name=f"xt{ch}")
                XTs[ch] = XT
                for g in range(NG):
                    nc.sync.dma_start(XT[:, g, :],
                                      xbs[:, g * 128:(g + 1) * 128],
                                      transpose=True)

            def emit_xpipe(ch):
                emit_cast(ch)
                emit_transposes(ch)

            # x chunks 0/1 start immediately; they don't need weights.
            emit_cast(0)
            emit_cast(1)
            emit_transposes(0)
            emit_transposes(1)

            # ---------------- weight dequant + transpose --------------
            for ot in range(O_TILES):
                r0 = ot * 128
                rows = min(128, OS - r0)
                oc = min(ot // 4, 2)
                o0 = OC_SLICES[oc][0]
                rc = r0 - o0
                ws_sb = wpool.tile([128, NG], F32, tag="ws")
                nc.scalar.dma_start(ws_sb[:rows, :], ws[r0:r0 + rows, :])
                for h in range(2):          # half-tiles: 16 groups each
                    g0 = h * 16
                    wqt = wpool.tile([128, NG * 32], I32, tag="wq",
                                     name=f"wq{ot}_{h}")
                    nc.scalar.dma_start(
                        wqt[:rows, :],
                        wq[r0:r0 + rows, g0 * 64:(g0 + 16) * 64])
                    # sign-extend nibbles: two wide strided DVE ops
                    wint = wpool.tile([128, IN_F // 2], I32, tag="wint",
                                      name=f"wint{ot}_{h}")
                    wr = wint[:rows].rearrange("p (b two) -> p b two", two=2)
                    nc.vector.tensor_scalar(
                        wr[:, :, 0], wqt[:rows, :], 28, 28,
                        mybir.AluOpType.logical_shift_left,
                        mybir.AluOpType.arith_shift_right)
                    nc.vector.tensor_scalar(
                        wr[:, :, 1], wqt[:rows, :], 24, 28,
                        mybir.AluOpType.logical_shift_left,
                        mybir.AluOpType.arith_shift_right)
                    # dequant: (wint * s_g) * xinv -> bf16, DVE/GpSimd split
                    wraw = wpool.tile([128, IN_F // 2], BF16, tag="wraw",
                                      name=f"wraw{ot}_{h}")
                    for j in range(16):
                        g = g0 + j
                        eng = nc.vector
                        blk = slice(j * 128, (j + 1) * 128)
                        eng.scalar_tensor_tensor(
                            wraw[:rows, blk], wint[:rows, blk],
                            ws_sb[:rows, g:g + 1],
                            xinv_sb[:rows, g * 128:(g + 1) * 128],
                            mybir.AluOpType.mult, mybir.AluOpType.mult)
                    # transpose 8 groups per PSUM bank, one wide evac each
                    for gb in range(0, 16, 8):
                        pt = wps.tile([128, 8, 128], BF16, tag="pt")
                        for j in range(8):
                            nc.tensor.transpose(
                                pt[:, j, :rows],
                                wraw[:rows, (gb + j) * 128:
                                     (gb + j + 1) * 128],
                                id_sb[:rows, :rows])
                        nc.scalar.activation(
                            WTs[oc][:, g0 + gb:g0 + gb + 8, rc:rc + rows],
                            pt[:, :, :rows],
                            mybir.ActivationFunctionType.Copy)

            # ---------------- main token loop (software pipelined) ----
            def emit_compute(ch):
                t0 = ch * TCH
                XT = XTs.pop(ch)
                for tt in range(TCH // 128):       # 4 token tiles per chunk
                    trow = t0 + tt * 128
                    accs = []
                    for oc in range(len(OC_SLICES)):
                        accs.append(aps.tile([128, 512], F32, tag="acc",
                                             name=f"acc{ch}_{tt}_{oc}"))
                    for g in range(NG):
                        lhsT = XT[:, g, tt * 128:(tt + 1) * 128]
                        for oc, (o0, o1) in enumerate(OC_SLICES):
                            nc.tensor.matmul(
                                accs[oc][:, :o1 - o0], lhsT,
                                WTs[oc][:, g, :],
                                start=(g == 0), stop=(g == NG - 1))
                    ystage = ypool.tile([128, OS], BF16, tag="ystage",
                                        name=f"ystage{ch}_{tt}")
                    for oc, (o0, o1) in enumerate(OC_SLICES):
                        nc.vector.tensor_tensor(
                            ystage[:, o0:o1], accs[oc][:, :o1 - o0],
                            bias_sb[:, o0:o1], mybir.AluOpType.add)
                    nc.gpsimd.dma_start(y[trow:trow + 128, :], ystage[:])

            for ch in range(2, N_CHUNK):
                emit_xpipe(ch)
                emit_compute(ch - 2)
            emit_compute(N_CHUNK - 2)
            emit_compute(N_CHUNK - 1)

    nc.compile()
    _NC_CACHE["nc"] = nc
    return nc


def make_in_maps(x, w_q_packed, w_scales, x_inv_s, bias):
    """Host-side shard + layout prep (reshapes/slices only)."""
    x2 = np.ascontiguousarray(
        np.asarray(x, dtype=np.float32).reshape(NT, IN_F))
    xinvb = np.ascontiguousarray(np.broadcast_to(
        np.asarray(x_inv_s).astype(ml_dtypes.bfloat16), (128, IN_F)))
    ident = np.eye(128, dtype=ml_dtypes.bfloat16)
    wq_full = np.asarray(w_q_packed, dtype=np.int32).reshape(OUT_F, NG * 64)
    ws_full = np.asarray(w_scales, dtype=np.float32).reshape(OUT_F, NG)
    bias_full = np.asarray(bias, dtype=np.float32)
    in_maps = []
    for c in range(N_CORES):
        o0 = c * OS
        in_maps.append({
            "x": x2,
            "wq": np.ascontiguousarray(wq_full[o0:o0 + OS]),
            "ws": np.ascontiguousarray(ws_full[o0:o0 + OS]),
            "xinvb": xinvb,
            "biasb": np.ascontiguousarray(
                np.broadcast_to(bias_full[o0:o0 + OS], (128, OS))
            ).astype(np.float32),
            "ident": ident,
        })
    return in_maps


def kernel(x, w_q_packed, w_scales, x_inv_s, bias):
    """Full inputs in, full output out; shards across 8 cores inside."""
    nc = build_nc()
    in_maps = make_in_maps(x, w_q_packed, w_scales, x_inv_s, bias)
    res = run_bass_kernel_spmd(nc, in_maps, list(range(N_CORES)),
                               trace=False)
    parts = [res.results[c]["y"] for c in range(N_CORES)]
    out = np.concatenate(parts, axis=-1).reshape(4, 2048, OUT_F)
    return out.astype(np.asarray(x).dtype)


# revision 7
# speedup vs baseline: 1.0333x; 1.0333x over previous
"""AWQ W4-packed linear layer (y = (x * x_inv_s) @ dequant(w).T + bias)
on 8 Trainium2 NeuronCores, tensor-parallel over out_features (no
collectives).

Per core (1376 of the 11008 out_features):
  * The kernel is tensor-engine bound: 64 token tiles x 32 groups x 1376
    outs of bf16 matmul = 2.82M PE cycles (~1.18 ms at 2.4 GHz). All
    other work is scheduled to hide under it; the previous version lost
    ~0.5 ms to a serial weight-dequant prologue.
  * Weight prep (pipelined, ~0.1 ms): per 128-row tile, one 8 KB/p int32
    DMA, two wide DVE shift ops sign-extend nibbles to int8, 32 fused
    scalar_tensor_tensor ops (split DVE/GpSimd) apply (w*s)*x_inv -> bf16,
    PE transposes 8 groups into one PSUM bank, one wide ACT copy
    evacuates to the SBUF-resident W^T [128 in, 32 g, out].
  * Activations: per 512-token chunk, a SWDGE cast-DMA converts x
    fp32->bf16 into DRAM staging; 32 xbar DMA-transposes (single HWDGE
    ring) produce X^T. Chunks 0-1 are issued at t=0 so the x pipe warms
    up concurrently with weight prep.
  * Matmul: per 128-token tile, three PSUM banks (N=512/512/352)
    accumulate 32 bf16 matmuls each; bias is added on PSUM evacuation
    (DVE, bf16 out) and a cast-DMA store writes the fp32 output shard.
"""
import sys

import numpy as np
import ml_dtypes

try:
    import concourse.bass as bass
except ImportError:  # fallback if PYTHONPATH lacks the repo
    for p in ("/opt/trn_rl_repo", "/root/.axon_site/_ro/trn_rl_repo"):
        if p not in sys.path:
            sys.path.append(p)
    import concourse.bass as bass

import concourse.bacc as bacc
import concourse.tile as tile
import concourse.mybir as mybir
from concourse.bass_utils import run_bass_kernel_spmd

F32 = mybir.dt.float32
BF16 = mybir.dt.bfloat16
I32 = mybir.dt.int32
I8 = mybir.dt.int8

OUT_F, IN_F, GROUP = 11008, 4096, 128
NG = IN_F // GROUP            # 32 quantization groups
NT = 8192                     # B*S tokens
N_CORES = 8
OS = OUT_F // N_CORES         # 1376 out features per core
TCH = 512                     # tokens per chunk
N_CHUNK = NT // TCH           # 16
O_TILES = (OS + 127) // 128   # 11 (last has 96 rows)
OC_SLICES = [(0, 512), (512, 1024), (1024, 1376)]

_NC_CACHE = {}


def build_nc():
    if "nc" in _NC_CACHE:
        return _NC_CACHE["nc"]
    nc = bacc.Bacc("TRN2", target_bir_lowering=False, debug=False,
                   num_devices=N_CORES)

    x = nc.dram_tensor("x", [NT, IN_F], F32, kind="ExternalInput").ap()
    wq = nc.dram_tensor("wq", [OS, NG * 64], I32, kind="ExternalInput").ap()
    ws = nc.dram_tensor("ws", [OS, NG], F32, kind="ExternalInput").ap()
    xinvb = nc.dram_tensor("xinvb", [128, IN_F], BF16,
                           kind="ExternalInput").ap()
    biasb = nc.dram_tensor("biasb", [128, OS], F32, kind="ExternalInput").ap()
    ident = nc.dram_tensor("ident", [128, 128], BF16,
                           kind="ExternalInput").ap()
    y = nc.dram_tensor("y", [NT, OS], F32, kind="ExternalOutput").ap()

    with tile.TileContext(nc) as tc:
        with (
            tc.tile_pool(name="cpool", bufs=1) as cpool,
            tc.tile_pool(name="wpool", bufs=2) as wpool,
            tc.tile_pool(name="xpool", bufs=2) as xpool,
            tc.tile_pool(name="dpool", bufs=2,
                         space=bass.MemorySpace.DRAM) as dpool,
            tc.tile_pool(name="ypool", bufs=2) as ypool,
            tc.tile_pool(name="wps", bufs=2,
                         space=bass.MemorySpace.PSUM) as wps,
            tc.tile_pool(name="aps", bufs=6,
                         space=bass.MemorySpace.PSUM) as aps,
        ):
            # ---------------- persistent tiles ----------------
            WTs = [cpool.tile([128, NG, o1 - o0], BF16, name=f"WT{i}")
                   for i, (o0, o1) in enumerate(OC_SLICES)]   # [i, g, o]
            id_sb = cpool.tile([128, 128], BF16)
            nc.scalar.dma_start(id_sb[:], ident[:])
            xinv_sb = cpool.tile([128, IN_F], BF16)
            nc.scalar.dma_start(xinv_sb[:], xinvb[:])
            bias_sb = cpool.tile([128, OS], F32)

            # ---------------- x pipeline helpers ----------------------
            XTs = {}
            xbss = {}

            def emit_cast(ch):
                t0 = ch * TCH
                xbs = dpool.tile([TCH, IN_F], BF16, tag="xbs",
                                 name=f"xbs{ch}")
                xbss[ch] = xbs
                nc.gpsimd.dma_start(xbs[:], x[t0:t0 + TCH, :])  # f32->bf16

            def emit_transposes(ch):
                xbs = xbss.pop(ch)
                XT = xpool.tile([128, NG, TCH], BF16, tag="xt",
                                name=f"xt{ch}")
                XTs[ch] = XT
                for g in range(NG):
                    nc.sync.dma_start(XT[:, g, :],
                                      xbs[:, g * 128:(g + 1) * 128],
                                      transpose=True)

            def emit_xpipe(ch):
                emit_cast(ch)
                emit_transposes(ch)

            # x chunk 0 starts immediately; it doesn't need weights.
            emit_xpipe(0)

            # ---------------- weight dequant + transpose --------------
            for ot in range(O_TILES):
                r0 = ot * 128
                rows = min(128, OS - r0)
                oc = min(ot // 4, 2)
                o0 = OC_SLICES[oc][0]
                rc = r0 - o0
                ws_sb = wpool.tile([128, NG], F32, tag="ws")
                nc.scalar.dma_start(ws_sb[:rows, :], ws[r0:r0 + rows, :])
                for h in range(2):          # half-tiles: 16 groups each
                    g0 = h * 16
                    wqt = wpool.tile([128, NG * 32], I32, tag="wq",
                                     name=f"wq{ot}_{h}")
                    nc.scalar.dma_start(
                        wqt[:rows, :],
                        wq[r0:r0 + rows, g0 * 64:(g0 + 16) * 64])
                    # sign-extend nibbles: two wide strided DVE ops
                    wint = wpool.tile([128, IN_F // 2], I32, tag="wint",
                                      name=f"wint{ot}_{h}")
                    wr = wint[:rows].rearrange("p (b two) -> p b two", two=2)
                    nc.vector.tensor_scalar(
                        wr[:, :, 0], wqt[:rows, :], 28, 28,
                        mybir.AluOpType.logical_shift_left,
                        mybir.AluOpType.arith_shift_right)
                    nc.vector.tensor_scalar(
                        wr[:, :, 1], wqt[:rows, :], 24, 28,
                        mybir.AluOpType.logical_shift_left,
                        mybir.AluOpType.arith_shift_right)
                    # dequant: (wint * s_g) * xinv -> bf16, DVE/GpSimd split
                    wraw = wpool.tile([128, IN_F // 2], BF16, tag="wraw",
                                      name=f"wraw{ot}_{h}")
                    for j in range(16):
                        g = g0 + j
                        eng = nc.vector
                        blk = slice(j * 128, (j + 1) * 128)
                        eng.scalar_tensor_tensor(
                            wraw[:rows, blk], wint[:rows, blk],
                            ws_sb[:rows, g:g + 1],
                            xinv_sb[:rows, g * 128:(g + 1) * 128],
                            mybir.AluOpType.mult, mybir.AluOpType.mult)
                    # transpose 8 groups per PSUM bank, one wide evac each
                    for gb in range(0, 16, 8):
                        pt = wps.tile([128, 8, 128], BF16, tag="pt")
                        for j in range(8):
                            nc.tensor.transpose(
                                pt[:, j, :rows],
                                wraw[:rows, (gb + j) * 128:
                                     (gb + j + 1) * 128],
                                id_sb[:rows, :rows])
                        nc.scalar.activation(
                            WTs[oc][:, g0 + gb:g0 + gb + 8, rc:rc + rows],
                            pt[:, :, :rows],
                            mybir.ActivationFunctionType.Copy)

            nc.scalar.dma_start(bias_sb[:], biasb[:])
            emit_xpipe(1)

            # ---------------- main token loop (software pipelined) ----
            def emit_compute(ch):
                t0 = ch * TCH
                XT = XTs.pop(ch)
                for tt in range(TCH // 128):       # 4 token tiles per chunk
                    trow = t0 + tt * 128
                    accs = []
                    for oc in range(len(OC_SLICES)):
                        accs.append(aps.tile([128, 512], F32, tag="acc",
                                             name=f"acc{ch}_{tt}_{oc}"))
                    for g in range(NG):
                        lhsT = XT[:, g, tt * 128:(tt + 1) * 128]
                        for oc, (o0, o1) in enumerate(OC_SLICES):
                            nc.tensor.matmul(
                                accs[oc][:, :o1 - o0], lhsT,
                                WTs[oc][:, g, :],
                                start=(g == 0), stop=(g == NG - 1))
                    ystage = ypool.tile([128, OS], F32, tag="ystage",
                                        name=f"ystage{ch}_{tt}")
                    for oc, (o0, o1) in enumerate(OC_SLICES):
                        nc.vector.tensor_tensor(
                            ystage[:, o0:o1], accs[oc][:, :o1 - o0],
                            bias_sb[:, o0:o1], mybir.AluOpType.add)
                    nc.sync.dma_start(y[trow:trow + 128, :], ystage[:])

            for ch in range(2, N_CHUNK):
                emit_xpipe(ch)
                emit_compute(ch - 2)
            emit_compute(N_CHUNK - 2)
            emit_compute(N_CHUNK - 1)

    nc.compile()
    _NC_CACHE["nc"] = nc
    return nc


def make_in_maps(x, w_q_packed, w_scales, x_inv_s, bias):
    """Host-side shard + layout prep (reshapes/slices only)."""
    x2 = np.ascontiguousarray(
        np.asarray(x, dtype=np.float32).reshape(NT, IN_F))
    xinvb = np.ascontiguousarray(np.broadcast_to(
        np.asarray(x_inv_s).astype(ml_dtypes.bfloat16), (128, IN_F)))
    ident = np.eye(128, dtype=ml_dtypes.bfloat16)
    wq_full = np.asarray(w_q_packed, dtype=np.int32).reshape(OUT_F, NG * 64)
    ws_full = np.asarray(w_scales, dtype=np.float32).reshape(OUT_F, NG)
    bias_full = np.asarray(bias, dtype=np.float32)
    in_maps = []
    for c in range(N_CORES):
        o0 = c * OS
        in_maps.append({
            "x": x2,
            "wq": np.ascontiguousarray(wq_full[o0:o0 + OS]),
            "ws": np.ascontiguousarray(ws_full[o0:o0 + OS]),
            "xinvb": xinvb,
            "biasb": np.ascontiguousarray(
                np.broadcast_to(bias_full[o0:o0 + OS], (128, OS))
            ).astype(np.float32),
            "ident": ident,
        })
    return in_maps


def kernel(x, w_q_packed, w_scales, x_inv_s, bias):
    """Full inputs in, full output out; shards across 8 cores inside."""
    nc = build_nc()
    in_maps = make_in_maps(x, w_q_packed, w_scales, x_inv_s, bias)
    res = run_bass_kernel_spmd(nc, in_maps, list(range(N_CORES)),
                               trace=False)
    parts = [res.results[c]["y"] for c in range(N_CORES)]
    out = np.concatenate(parts, axis=-1).reshape(4, 2048, OUT_F)
    return out.astype(np.asarray(x).dtype)


# revision 9
# speedup vs baseline: 1.0702x; 1.0356x over previous
"""AWQ W4-packed linear layer (y = (x * x_inv_s) @ dequant(w).T + bias)
on 8 Trainium2 NeuronCores, tensor-parallel over out_features (no
collectives).

Per core (1376 of the 11008 out_features):
  * The kernel is tensor-engine bound: 64 token tiles x 32 groups x 1376
    outs of bf16 matmul = 2.82M PE cycles (~1.18 ms at 2.4 GHz). All
    other work is scheduled to hide under it; the previous version lost
    ~0.5 ms to a serial weight-dequant prologue.
  * Weight prep (pipelined, ~0.1 ms): per 128-row tile, one 8 KB/p int32
    DMA, two wide DVE shift ops sign-extend nibbles to int8, 32 fused
    scalar_tensor_tensor ops (split DVE/GpSimd) apply (w*s)*x_inv -> bf16,
    PE transposes 8 groups into one PSUM bank, one wide ACT copy
    evacuates to the SBUF-resident W^T [128 in, 32 g, out].
  * Activations: per 512-token chunk, a SWDGE cast-DMA converts x
    fp32->bf16 into DRAM staging; 32 xbar DMA-transposes (single HWDGE
    ring) produce X^T. Chunks 0-1 are issued at t=0 so the x pipe warms
    up concurrently with weight prep.
  * Matmul: per 128-token tile, three PSUM banks (N=512/512/352)
    accumulate 32 bf16 matmuls each; bias is added on PSUM evacuation
    (DVE, bf16 out) and a cast-DMA store writes the fp32 output shard.
"""
import sys

import numpy as np
import ml_dtypes

try:
    import concourse.bass as bass
except ImportError:  # fallback if PYTHONPATH lacks the repo
    for p in ("/opt/trn_rl_repo", "/root/.axon_site/_ro/trn_rl_repo"):
        if p not in sys.path:
            sys.path.append(p)
    import concourse.bass as bass

import concourse.bacc as bacc
import concourse.tile as tile
import concourse.mybir as mybir
from concourse.bass_utils import run_bass_kernel_spmd

F32 = mybir.dt.float32
BF16 = mybir.dt.bfloat16
I32 = mybir.dt.int32
I8 = mybir.dt.int8

OUT_F, IN_F, GROUP = 11008, 4096, 128
NG = IN_F // GROUP            # 32 quantization groups
NT = 8192                     # B*S tokens
N_CORES = 8
OS = OUT_F // N_CORES         # 1376 out features per core
TCH = 512                     # tokens per chunk
N_CHUNK = NT // TCH           # 16
O_TILES = (OS + 127) // 128   # 11 (last has 96 rows)
OC_SLICES = [(0, 512), (512, 1024), (1024, 1376)]

_NC_CACHE = {}


def build_nc():
    if "nc" in _NC_CACHE:
        return _NC_CACHE["nc"]
    nc = bacc.Bacc("TRN2", target_bir_lowering=False, debug=False,
                   num_devices=N_CORES)

    x = nc.dram_tensor("x", [NT, IN_F], F32, kind="ExternalInput").ap()
    wq = nc.dram_tensor("wq", [OS, NG * 64], I32, kind="ExternalInput").ap()
    ws = nc.dram_tensor("ws", [OS, NG], F32, kind="ExternalInput").ap()
    xinvb = nc.dram_tensor("xinvb", [128, IN_F], BF16,
                           kind="ExternalInput").ap()
    biasb = nc.dram_tensor("biasb", [128, OS], F32, kind="ExternalInput").ap()
    ident = nc.dram_tensor("ident", [128, 128], BF16,
                           kind="ExternalInput").ap()
    y = nc.dram_tensor("y", [NT, OS], F32, kind="ExternalOutput").ap()

    with tile.TileContext(nc) as tc:
        with (
            tc.tile_pool(name="cpool", bufs=1) as cpool,
            tc.tile_pool(name="wpool", bufs=2) as wpool,
            tc.tile_pool(name="xpool", bufs=2) as xpool,
            tc.tile_pool(name="dpool", bufs=2,
                         space=bass.MemorySpace.DRAM) as dpool,
            tc.tile_pool(name="ypool", bufs=2) as ypool,
            tc.tile_pool(name="wps", bufs=2,
                         space=bass.MemorySpace.PSUM) as wps,
            tc.tile_pool(name="aps", bufs=6,
                         space=bass.MemorySpace.PSUM) as aps,
        ):
            # ---------------- persistent tiles ----------------
            WTs = [cpool.tile([128, NG, o1 - o0], BF16, name=f"WT{i}")
                   for i, (o0, o1) in enumerate(OC_SLICES)]   # [i, g, o]
            id_sb = cpool.tile([128, 128], BF16)
            nc.scalar.dma_start(id_sb[:], ident[:])
            xinv_sb = cpool.tile([128, IN_F], BF16)
            nc.scalar.dma_start(xinv_sb[:], xinvb[:])
            bias_sb = cpool.tile([128, OS], F32)

            # ---------------- x pipeline helpers ----------------------
            XTs = {}
            xbss = {}

            def emit_cast(ch):
                t0 = ch * TCH
                xbs = dpool.tile([TCH, IN_F], BF16, tag="xbs",
                                 name=f"xbs{ch}")
                xbss[ch] = xbs
                nc.gpsimd.dma_start(xbs[:], x[t0:t0 + TCH, :])  # f32->bf16

            def emit_transposes(ch):
                xbs = xbss.pop(ch)
                XT = xpool.tile([128, NG, TCH], BF16, tag="xt",
                                name=f"xt{ch}")
                XTs[ch] = XT
                for g in range(NG):
                    nc.sync.dma_start(XT[:, g, :],
                                      xbs[:, g * 128:(g + 1) * 128],
                                      transpose=True)

            def emit_xpipe(ch):
                emit_cast(ch)
                emit_transposes(ch)

            # x chunks 0/1 start immediately; they don't need weights.
            emit_xpipe(0)
            emit_xpipe(1)

            # ---------------- weight dequant + transpose --------------
            # wq/ws loads prefetch 2 row-tiles ahead of the consume
            # pipeline on the scalar queue; evacs queue behind them.
            ws_sbs = {}
            wqts = {}

            def trig(ot):
                r0 = ot * 128
                rows = min(128, OS - r0)
                ws_sb = wpool.tile([128, NG], F32, tag="ws", bufs=4)
                ws_sbs[ot] = ws_sb
                nc.scalar.dma_start(ws_sb[:rows, :], ws[r0:r0 + rows, :])
                for h in range(2):
                    wqt = wpool.tile([128, NG * 32], I32, tag="wq", bufs=4,
                                     name=f"wq{ot}_{h}")
                    wqts[(ot, h)] = wqt
                    nc.scalar.dma_start(
                        wqt[:rows, :],
                        wq[r0:r0 + rows, h * 1024:(h + 1) * 1024])

            trig(0)
            trig(1)
            for ot in range(O_TILES):
                if ot + 2 < O_TILES:
                    trig(ot + 2)
                r0 = ot * 128
                rows = min(128, OS - r0)
                oc = min(ot // 4, 2)
                o0 = OC_SLICES[oc][0]
                rc = r0 - o0
                ws_sb = ws_sbs.pop(ot)
                for h in range(2):          # half-tiles: 16 groups each
                    g0 = h * 16
                    wqt = wqts.pop((ot, h))
                    # sign-extend nibbles: two wide strided DVE ops
                    wint = wpool.tile([128, IN_F // 2], I32, tag="wint", bufs=1,
                                      name=f"wint{ot}_{h}")
                    wr = wint[:rows].rearrange("p (b two) -> p b two", two=2)
                    nc.vector.tensor_scalar(
                        wr[:, :, 0], wqt[:rows, :], 28, 28,
                        mybir.AluOpType.logical_shift_left,
                        mybir.AluOpType.arith_shift_right)
                    nc.vector.tensor_scalar(
                        wr[:, :, 1], wqt[:rows, :], 24, 28,
                        mybir.AluOpType.logical_shift_left,
                        mybir.AluOpType.arith_shift_right)
                    # dequant: (wint * s_g) * xinv -> bf16 on DVE
                    wraw = wpool.tile([128, IN_F // 2], BF16, tag="wraw",
                                      name=f"wraw{ot}_{h}")
                    for j in range(16):
                        g = g0 + j
                        blk = slice(j * 128, (j + 1) * 128)
                        nc.vector.scalar_tensor_tensor(
                            wraw[:rows, blk], wint[:rows, blk],
                            ws_sb[:rows, g:g + 1],
                            xinv_sb[:rows, g * 128:(g + 1) * 128],
                            mybir.AluOpType.mult, mybir.AluOpType.mult)
                    # transpose 8 groups per PSUM bank, one wide evac each
                    for gb in range(0, 16, 8):
                        pt = wps.tile([128, 8, 128], BF16, tag="pt")
                        for j in range(8):
                            nc.tensor.transpose(
                                pt[:, j, :rows],
                                wraw[:rows, (gb + j) * 128:
                                     (gb + j + 1) * 128],
                                id_sb[:rows, :rows])
                        nc.scalar.activation(
                            WTs[oc][:, g0 + gb:g0 + gb + 8, rc:rc + rows],
                            pt[:, :, :rows],
                            mybir.ActivationFunctionType.Copy)

            nc.scalar.dma_start(bias_sb[:], biasb[:])

            # ---------------- main token loop (software pipelined) ----
            def emit_compute(ch):
                t0 = ch * TCH
                XT = XTs.pop(ch)
                for tt in range(TCH // 128):       # 4 token tiles per chunk
                    trow = t0 + tt * 128
                    accs = []
                    for oc in range(len(OC_SLICES)):
                        accs.append(aps.tile([128, 512], F32, tag="acc",
                                             name=f"acc{ch}_{tt}_{oc}"))
                    for g in range(NG):
                        lhsT = XT[:, g, tt * 128:(tt + 1) * 128]
                        for oc, (o0, o1) in enumerate(OC_SLICES):
                            nc.tensor.matmul(
                                accs[oc][:, :o1 - o0], lhsT,
                                WTs[oc][:, g, :],
                                start=(g == 0), stop=(g == NG - 1))
                    ystage = ypool.tile([128, OS], F32, tag="ystage",
                                        name=f"ystage{ch}_{tt}")
                    for oc, (o0, o1) in enumerate(OC_SLICES):
                        nc.vector.tensor_tensor(
                            ystage[:, o0:o1], accs[oc][:, :o1 - o0],
                            bias_sb[:, o0:o1], mybir.AluOpType.add)
                    nc.sync.dma_start(y[trow:trow + 128, :], ystage[:])

            for ch in range(2, N_CHUNK):
                emit_xpipe(ch)
                emit_compute(ch - 2)
            emit_compute(N_CHUNK - 2)
            emit_compute(N_CHUNK - 1)

    nc.compile()
    _NC_CACHE["nc"] = nc
    return nc


def make_in_maps(x, w_q_packed, w_scales, x_inv_s, bias):
    """Host-side shard + layout prep (reshapes/slices only)."""
    x2 = np.ascontiguousarray(
        np.asarray(x, dtype=np.float32).reshape(NT, IN_F))
    xinvb = np.ascontiguousarray(np.broadcast_to(
        np.asarray(x_inv_s).astype(ml_dtypes.bfloat16), (128, IN_F)))
    ident = np.eye(128, dtype=ml_dtypes.bfloat16)
    wq_full = np.asarray(w_q_packed, dtype=np.int32).reshape(OUT_F, NG * 64)
    ws_full = np.asarray(w_scales, dtype=np.float32).reshape(OUT_F, NG)
    bias_full = np.asarray(bias, dtype=np.float32)
    in_maps = []
    for c in range(N_CORES):
        o0 = c * OS
        in_maps.append({
            "x": x2,
            "wq": np.ascontiguousarray(wq_full[o0:o0 + OS]),
            "ws": np.ascontiguousarray(ws_full[o0:o0 + OS]),
            "xinvb": xinvb,
            "biasb": np.ascontiguousarray(
                np.broadcast_to(bias_full[o0:o0 + OS], (128, OS))
            ).astype(np.float32),
            "ident": ident,
        })
    return in_maps


def kernel(x, w_q_packed, w_scales, x_inv_s, bias):
    """Full inputs in, full output out; shards across 8 cores inside."""
    nc = build_nc()
    in_maps = make_in_maps(x, w_q_packed, w_scales, x_inv_s, bias)
    res = run_bass_kernel_spmd(nc, in_maps, list(range(N_CORES)),
                               trace=False)
    parts = [res.results[c]["y"] for c in range(N_CORES)]
    out = np.concatenate(parts, axis=-1).reshape(4, 2048, OUT_F)
    return out.astype(np.asarray(x).dtype)
